# revision 6
# baseline (speedup 1.0000x reference)
"""ConvAttnLSTM cell TRN2 Bass kernel — batch-parallel over 8 NeuronCores.

Layouts (per core, BL=64 batches):
  conv layout:      [channels e (partitions), (b, yx) free]
  attention layout: [(b%16, n) partitions=128, d=de*64+yx free]
Phases per 32-batch group: A (convs+gates), B (attention), C1 (out conv+LN stats),
C2 (normalize+gating). Head-layout transposes ride through DRAM (out_k/out_v
slot 7 + small scratch), which are required outputs anyway.
"""
import math
import numpy as np
import ml_dtypes

import concourse.bacc as bacc
import concourse.bass as bass
import concourse.tile as tile
from concourse import mybir
from concourse.bass_utils import run_bass_kernel_spmd

f32 = mybir.dt.float32
bf16 = mybir.dt.bfloat16
AL = mybir.AluOpType
AF = mybir.ActivationFunctionType

B, C, E, H, W = 512, 64, 64, 8, 8
NH, MEM, HD = 8, 8, 8
THD = H * W * HD            # 512
YX = H * W                  # 64
NC_ = 8                     # cores
BL = B // NC_               # 64 batches/core
NEG = -1e30
LN_EPS = 1e-5


def _build(ln_trivial):
    nc = bacc.Bacc(None, target_bir_lowering=False)
    x_in = nc.dram_tensor("x", [BL, C, YX], f32, kind="ExternalInput")
    h_in = nc.dram_tensor("h", [BL, E, YX], f32, kind="ExternalInput")
    c_in = nc.dram_tensor("cc", [BL, E, YX], f32, kind="ExternalInput")
    ck_in = nc.dram_tensor("ck", [BL, MEM, NH, THD], f32, kind="ExternalInput")
    cv_in = nc.dram_tensor("cv", [BL, MEM, NH, THD], f32, kind="ExternalInput")
    mp_in = nc.dram_tensor("mp", [BL, NH, MEM], f32, kind="ExternalInput")
    w1_in = nc.dram_tensor("w1", [9, C + E, 5 * E], bf16, kind="ExternalInput")
    wp_in = nc.dram_tensor("wp", [9, C, 3 * E], bf16, kind="ExternalInput")
    wo_in = nc.dram_tensor("wo", [9, E, E], bf16, kind="ExternalInput")
    cb_in = nc.dram_tensor("cb", [3, 128], f32, kind="ExternalInput")
    pb_in = nc.dram_tensor("pb", [2, 128], f32, kind="ExternalInput")
    ob_in = nc.dram_tensor("ob", [1, E], f32, kind="ExternalInput")
    posm_in = nc.dram_tensor("posm", [128, MEM - 1, THD], f32, kind="ExternalInput")
    pos7_in = nc.dram_tensor("pos7", [E, YX], f32, kind="ExternalInput")
    if not ln_trivial:
        lnw_in = nc.dram_tensor("lnw", [E, YX], f32, kind="ExternalInput")
        lnb_in = nc.dram_tensor("lnb", [E, YX], f32, kind="ExternalInput")

    ho = nc.dram_tensor("h_next", [BL, E, YX], f32, kind="ExternalOutput")
    co = nc.dram_tensor("c_next", [BL, E, YX], f32, kind="ExternalOutput")
    ko = nc.dram_tensor("k_out", [BL, MEM, NH, THD], f32, kind="ExternalOutput")
    vo = nc.dram_tensor("v_out", [BL, MEM, NH, THD], f32, kind="ExternalOutput")
    qs = nc.dram_tensor("q_scr", [BL, NH, THD], f32, kind="ExternalOutput")
    ats = nc.dram_tensor("a_scr", [BL, NH, THD], bf16, kind="ExternalOutput")

    from contextlib import ExitStack
    with tile.TileContext(nc) as tc, ExitStack() as es:
        cst = es.enter_context(tc.tile_pool(name="cst", bufs=1))
        ap_ = es.enter_context(tc.tile_pool(name="apool", bufs=2))
        keep = es.enter_context(tc.tile_pool(name="keep", bufs=4))
        batt = es.enter_context(tc.tile_pool(name="batt", bufs=2))
        cp_ = es.enter_context(tc.tile_pool(name="cpool", bufs=2))
        pp = es.enter_context(tc.tile_pool(name="pp", bufs=2, space="PSUM"))

        # ---- constants ----
        w1_sb = cst.tile([C + E, 9, 5 * E], bf16)
        nc.sync.dma_start(out=w1_sb, in_=w1_in.ap().transpose([1, 0, 2]))
        wp_sb = cst.tile([C, 9, 3 * E], bf16)
        nc.sync.dma_start(out=wp_sb, in_=wp_in.ap().transpose([1, 0, 2]))
        wo_sb = cst.tile([E, 9, E], bf16)
        nc.sync.dma_start(out=wo_sb, in_=wo_in.ap().transpose([1, 0, 2]))
        cb_sb = cst.tile([128, 3], f32)
        nc.sync.dma_start(out=cb_sb, in_=cb_in.ap().transpose([1, 0]))
        pb_sb = cst.tile([128, 2], f32)
        nc.sync.dma_start(out=pb_sb, in_=pb_in.ap().transpose([1, 0]))
        ob_sb = cst.tile([E, 1], f32)
        nc.sync.dma_start(out=ob_sb, in_=ob_in.ap().transpose([1, 0]))
        posm_sb = cst.tile([128, MEM - 1, THD], f32)
        nc.sync.dma_start(out=posm_sb, in_=posm_in.ap())
        pos7_sb = cst.tile([E, YX], f32)
        nc.sync.dma_start(out=pos7_sb, in_=pos7_in.ap())
        if not ln_trivial:
            lnw_sb = cst.tile([E, YX], f32)
            nc.sync.dma_start(out=lnw_sb, in_=lnw_in.ap())
            lnb_sb = cst.tile([E, YX], f32)
            nc.sync.dma_start(out=lnb_sb, in_=lnb_in.ap())
        mp_sb = cst.tile([128, 4, MEM], f32)
        for T in range(4):
            nc.sync.dma_start(out=mp_sb[:, T, :], in_=mp_in.ap()[16 * T:16 * T + 16])
        eps_sb = cst.tile([E, 1], f32)
        nc.vector.memset(eps_sb, LN_EPS)
        ones_sb = cst.tile([E, E], f32)
        nc.vector.memset(ones_sb, 1.0 / (E * YX))

        comb = cst.tile([C + E, BL, YX], f32)
        nc.sync.dma_start(out=comb[0:C], in_=x_in.ap().transpose([1, 0, 2]))
        nc.sync.dma_start(out=comb[C:C + E], in_=h_in.ap().transpose([1, 0, 2]))
        c_big = cst.tile([E, BL, YX], f32)
        nc.sync.dma_start(out=c_big, in_=c_in.ap().transpose([1, 0, 2]))

        def bc_free(t, reps):
            # broadcast [P, YX] tile along a new middle (batch) dim
            return bass.AP(tensor=t.tensor, offset=t.offset,
                           ap=[list(t.ap[0]), [0, reps], list(t.ap[1])])

        a_keep, o_keep, cn_keep, out_keep, mu_keep, ri_keep = {}, {}, {}, {}, {}, {}

        def phaseA(t):
            b0 = 8 * t
            padded = ap_.tile([C + E, 8, 10, 10], bf16, tag="pad1")
            nc.gpsimd.memset(padded, 0)
            nc.gpsimd.tensor_copy(
                out=padded[:, :, 1:9, 1:9],
                in_=comb[:, b0:b0 + 8, :].rearrange("p b (y x) -> p b y x", x=8))
            # conv1: chunks [i|f], [o|g], [a]
            sif = ap_.tile([128, 512], f32, tag="sif")
            og = ap_.tile([128, 512], f32, tag="og")
            a_k = keep.tile([E, 512], f32, tag="ak")
            for ci, (m0, m1) in enumerate(((0, 128), (128, 256), (256, 320))):
                ps = pp.tile([m1 - m0, 512], f32, tag="ps_c1")
                for s in range(9):
                    ky, kx = s // 3, s % 3
                    nc.tensor.matmul(out=ps, lhsT=w1_sb[:, s, m0:m1],
                                     rhs=padded[:, :, ky:ky + 8, kx:kx + 8],
                                     start=(s == 0), stop=(s == 8))
                if ci == 0:
                    nc.scalar.activation(out=sif, in_=ps, func=AF.Sigmoid,
                                         bias=cb_sb[:, 0:1])
                elif ci == 1:
                    nc.scalar.activation(out=o_keep[t], in_=ps[0:64], func=AF.Sigmoid,
                                         bias=cb_sb[0:64, 1:2])
                    nc.scalar.activation(out=og[64:128], in_=ps[64:128], func=AF.Tanh,
                                         bias=cb_sb[64:128, 1:2])
                else:
                    nc.scalar.activation(out=a_k, in_=ps, func=AF.Sigmoid,
                                         bias=cb_sb[0:64, 2:3])
            a_keep[t] = a_k
            f_t = ap_.tile([E, 512], f32, tag="ft")
            nc.sync.dma_start(out=f_t, in_=sif[64:128, :])
            g_t = ap_.tile([E, 512], f32, tag="gt")
            nc.sync.dma_start(out=g_t, in_=og[64:128, :])
            t1 = ap_.tile([E, 512], f32, tag="t1")
            nc.vector.tensor_mul(t1, sif[0:64], g_t)
            t2 = ap_.tile([E, 512], f32, tag="t2")
            nc.vector.tensor_mul(t2, f_t, c_big[:, b0:b0 + 8, :].rearrange("p b x -> p (b x)"))
            cn = keep.tile([E, 512], f32, tag="cn")
            nc.vector.tensor_add(cn, t1, t2)
            cn_keep[t] = cn
            # proj conv: [k|q], [v]
            kq = ap_.tile([128, 512], f32, tag="kq")
            v_t = ap_.tile([E, 512], f32, tag="vt")
            for ci, (m0, m1) in enumerate(((0, 128), (128, 192))):
                ps = pp.tile([m1 - m0, 512], f32, tag="ps_pj")
                for s in range(9):
                    ky, kx = s // 3, s % 3
                    nc.tensor.matmul(out=ps, lhsT=wp_sb[:, s, m0:m1],
                                     rhs=padded[0:C, :, ky:ky + 8, kx:kx + 8],
                                     start=(s == 0), stop=(s == 8))
                if ci == 0:
                    nc.scalar.activation(out=kq, in_=ps, func=AF.Identity,
                                         bias=pb_sb[:, 0:1])
                else:
                    nc.scalar.activation(out=v_t, in_=ps, func=AF.Identity,
                                         bias=pb_sb[0:64, 1:2])
            k7 = ap_.tile([E, 512], f32, tag="k7")
            nc.vector.tensor_tensor(out=k7, in0=kq[0:64], in1=bc_free(pos7_sb, 8), op=AL.add)
            # slot-7 / q writebacks (head-major in DRAM)
            dst_k7 = ko.ap()[b0:b0 + 8, 7:8, :, :].rearrange(
                "b o n (de yx) -> o n de b yx", de=HD)
            nc.sync.dma_start(out=dst_k7, in_=k7.rearrange("p (b yx) -> p b yx", b=8))
            dst_v7 = vo.ap()[b0:b0 + 8, 7:8, :, :].rearrange(
                "b o n (de yx) -> o n de b yx", de=HD)
            nc.sync.dma_start(out=dst_v7, in_=v_t.rearrange("p (b yx) -> p b yx", b=8))
            dst_q = qs.ap()[b0:b0 + 8].rearrange("b n (de yx) -> n de b yx", de=HD)
            nc.sync.dma_start(out=dst_q, in_=kq[64:128].rearrange("p (b yx) -> p b yx", b=8))

        def phaseB(T):
            b0 = 16 * T
            katt = batt.tile([128, MEM, THD], f32, tag="katt")
            for m in range(7):
                nc.sync.dma_start(out=katt[:, m, :],
                                  in_=ck_in.ap()[b0:b0 + 16, 1 + m])
            nc.sync.dma_start(out=katt[:, 7, :],
                              in_=ko.ap()[b0:b0 + 16, 7:8].rearrange("b o n d -> (b o) n d"))
            nc.vector.tensor_add(katt[:, 0:7, :], katt[:, 0:7, :], posm_sb)
            for m in range(7):
                nc.sync.dma_start(out=ko.ap()[b0:b0 + 16, m], in_=katt[:, m, :])
            vatt = batt.tile([128, MEM, THD], f32, tag="vatt", bufs=1)
            for m in range(7):
                nc.sync.dma_start(out=vatt[:, m, :],
                                  in_=cv_in.ap()[b0:b0 + 16, 1 + m])
            nc.sync.dma_start(out=vatt[:, 7, :],
                              in_=vo.ap()[b0:b0 + 16, 7:8].rearrange("b o n d -> (b o) n d"))
            for m in range(7):
                nc.sync.dma_start(out=vo.ap()[b0:b0 + 16, m], in_=vatt[:, m, :])
            qatt = batt.tile([128, THD], f32, tag="qatt")
            nc.sync.dma_start(out=qatt, in_=qs.ap()[b0:b0 + 16])
            sc = batt.tile([128, MEM], f32, tag="sc")
            dump = batt.tile([128, THD], f32, tag="dump", bufs=1)
            for m in range(MEM):
                nc.vector.scalar_tensor_tensor(out=dump, in0=katt[:, m, :], scalar=1.0,
                                               in1=qatt, op0=AL.mult, op1=AL.mult,
                                               accum_out=sc[:, m:m + 1])
            nc.vector.tensor_add(sc, sc, mp_sb[:, T, :])
            e_t = batt.tile([128, MEM], f32, tag="et")
            nc.scalar.activation(out=e_t, in_=sc, func=AF.Exp)
            se = batt.tile([128, 1], f32, tag="se")
            nc.vector.reduce_sum(out=se, in_=e_t, axis=mybir.AxisListType.X)
            rse = batt.tile([128, 1], f32, tag="rse")
            nc.vector.reciprocal(out=rse, in_=se)
            acc = batt.tile([128, THD], f32, tag="acc", bufs=1)
            nc.vector.tensor_scalar_mul(out=acc, in0=vatt[:, 0, :], scalar1=e_t[:, 0:1])
            for m in range(1, MEM):
                nc.vector.scalar_tensor_tensor(out=acc, in0=vatt[:, m, :],
                                               scalar=e_t[:, m:m + 1], in1=acc,
                                               op0=AL.mult, op1=AL.add)
            attn16 = batt.tile([128, THD], bf16, tag="attn")
            nc.vector.tensor_scalar_mul(out=attn16, in0=acc, scalar1=rse)
            nc.sync.dma_start(out=ats.ap()[b0:b0 + 16], in_=attn16)

        def phaseC1(t):
            b0 = 8 * t
            atc = cp_.tile([E, 8, YX], bf16, tag="atc")
            nc.sync.dma_start(out=atc,
                              in_=ats.ap()[b0:b0 + 8].rearrange("b n (de yx) -> (n de) b yx", de=HD))
            pad2 = cp_.tile([E, 8, 10, 10], bf16, tag="pad2")
            nc.gpsimd.memset(pad2, 0)
            nc.gpsimd.tensor_copy(out=pad2[:, :, 1:9, 1:9],
                                  in_=atc.rearrange("p b (y x) -> p b y x", x=8))
            ps = pp.tile([E, 512], f32, tag="ps_o")
            for s in range(9):
                ky, kx = s // 3, s % 3
                nc.tensor.matmul(out=ps, lhsT=wo_sb[:, s, :],
                                 rhs=pad2[:, :, ky:ky + 8, kx:kx + 8],
                                 start=(s == 0), stop=(s == 8))
            out_t = keep.tile([E, 512], f32, tag="out")
            nc.vector.scalar_tensor_tensor(
                out=out_t, in0=ps, scalar=ob_sb, op0=AL.add,
                in1=comb[0:C, b0:b0 + 8, :].rearrange("p b x -> p (b x)"), op1=AL.add)
            out_keep[t] = out_t
            sq = cp_.tile([E, 512], f32, tag="sq")
            nc.scalar.activation(out=sq, in_=out_t, func=AF.Square)
            stat = cp_.tile([E, 16], f32, tag="stat")
            nc.vector.reduce_sum(out=stat[:, 0:8],
                                 in_=out_t.rearrange("p (b x) -> p b x", b=8),
                                 axis=mybir.AxisListType.X)
            nc.vector.reduce_sum(out=stat[:, 8:16],
                                 in_=sq.rearrange("p (b x) -> p b x", b=8),
                                 axis=mybir.AxisListType.X)
            ps_st = pp.tile([E, 16], f32, tag="ps_st")
            nc.tensor.matmul(out=ps_st, lhsT=ones_sb, rhs=stat, start=True, stop=True)
            mu = keep.tile([E, 8], f32, tag="mu")
            nc.vector.tensor_copy(out=mu, in_=ps_st[:, 0:8])
            mu_keep[t] = mu
            musq = cp_.tile([E, 8], f32, tag="musq")
            nc.vector.tensor_mul(musq, mu, mu)
            var_t = cp_.tile([E, 8], f32, tag="var")
            nc.vector.tensor_tensor(out=var_t, in0=ps_st[:, 8:16], in1=musq, op=AL.subtract)
            sd_t = cp_.tile([E, 8], f32, tag="sd")
            nc.scalar.activation(out=sd_t, in_=var_t, func=AF.Sqrt, bias=eps_sb)
            ri = keep.tile([E, 8], f32, tag="ri")
            nc.vector.reciprocal(out=ri, in_=sd_t)
            ri_keep[t] = ri

        def phaseC2(t):
            b0 = 8 * t
            out_t, mu, ri = out_keep[t], mu_keep[t], ri_keep[t]
            ov = out_t.rearrange("p (b x) -> p b x", b=8)
            for b in range(8):
                nc.vector.tensor_scalar(out=ov[:, b, :], in0=ov[:, b, :],
                                        scalar1=mu[:, b:b + 1], scalar2=ri[:, b:b + 1],
                                        op0=AL.subtract, op1=AL.mult)
            if not ln_trivial:
                nc.vector.tensor_tensor(out=out_t, in0=out_t, in1=bc_free(lnw_sb, 8), op=AL.mult)
                nc.vector.tensor_tensor(out=out_t, in0=out_t, in1=bc_free(lnb_sb, 8), op=AL.add)
            th = cp_.tile([E, 512], f32, tag="th")
            nc.scalar.activation(out=th, in_=out_t, func=AF.Tanh)
            nc.vector.tensor_mul(th, a_keep[t], th)
            c2 = cp_.tile([E, 512], f32, tag="c2")
            nc.vector.tensor_add(c2, cn_keep[t], th)
            nc.sync.dma_start(out=co.ap()[b0:b0 + 8].transpose([1, 0, 2]),
                              in_=c2.rearrange("p (b x) -> p b x", b=8))
            th2 = cp_.tile([E, 512], f32, tag="th")
            nc.scalar.activation(out=th2, in_=c2, func=AF.Tanh)
            nc.vector.tensor_mul(th2, o_keep[t], th2)
            nc.sync.dma_start(out=ho.ap()[b0:b0 + 8].transpose([1, 0, 2]),
                              in_=th2.rearrange("p (b x) -> p b x", b=8))

        for t in range(8):
            o_keep[t] = keep.tile([E, 512], f32, tag="ok", name=f"ok{t}")

        for g in range(2):
            for t in range(4 * g, 4 * g + 4):
                phaseA(t)
            for T in range(2 * g, 2 * g + 2):
                phaseB(T)
            for t in range(4 * g, 4 * g + 4):
                phaseC1(t)
            for t in range(4 * g, 4 * g + 4):
                phaseC2(t)

    nc.compile()
    return nc


_BUILT = {}


def _get_nc(ln_trivial):
    if ln_trivial not in _BUILT:
        _BUILT[ln_trivial] = _build(ln_trivial)
    return _BUILT[ln_trivial]


def _prep(inputs):
    g = {k: np.asarray(v) for k, v in inputs.items()}
    conv_w = g["conv_w"].astype(np.float32)
    proj_w = g["proj_w"].astype(np.float32).copy()
    proj_b = g["proj_b"].astype(np.float32).copy()
    out_w = g["out_w"].astype(np.float32)
    scale = 1.0 / math.sqrt(THD)
    proj_w[E:2 * E] *= scale
    proj_b[E:2 * E] *= scale

    w1 = np.ascontiguousarray(conv_w.transpose(2, 3, 1, 0).reshape(9, C + E, 5 * E)).astype(ml_dtypes.bfloat16)
    wp = np.ascontiguousarray(proj_w.transpose(2, 3, 1, 0).reshape(9, C, 3 * E)).astype(ml_dtypes.bfloat16)
    wo = np.ascontiguousarray(out_w.transpose(2, 3, 1, 0).reshape(9, E, E)).astype(ml_dtypes.bfloat16)

    cb = np.zeros((3, 128), np.float32)
    cbv = g["conv_b"].astype(np.float32)
    cb[0, :] = cbv[0:128]
    cb[1, :] = cbv[128:256]
    cb[2, :64] = cbv[256:320]
    pb = np.zeros((2, 128), np.float32)
    pb[0, :] = proj_b[0:128]
    pb[1, :64] = proj_b[128:192]
    ob = g["out_b"].astype(np.float32).reshape(1, E)

    pos_w = g["pos_w"].astype(np.float32)
    pw7 = pos_w[:7].reshape(7, NH, THD)                     # [m, n, d]
    posm = np.ascontiguousarray(
        np.broadcast_to(pw7.transpose(1, 0, 2)[None], (16, NH, 7, THD))
    ).reshape(128, 7, THD)
    pos7 = pos_w[7].reshape(E, YX).copy()

    mask = g["attn_mask"].reshape(B, NH, MEM)
    mp = np.where(mask, np.float32(NEG), np.float32(0.0)).astype(np.float32)
    mp[:, :, MEM - 1] = 5.0
    mp += g["pos_b"].astype(np.float32).T[None]

    ln_w = g["ln_w"].astype(np.float32)
    ln_b = g["ln_b"].astype(np.float32)
    ln_trivial = bool(np.all(ln_w == 1.0) and np.all(ln_b == 0.0))

    shared = dict(w1=w1, wp=wp, wo=wo, cb=cb, pb=pb, ob=ob, posm=posm, pos7=pos7)
    if not ln_trivial:
        shared["lnw"] = ln_w.reshape(E, YX)
        shared["lnb"] = ln_b.reshape(E, YX)

    x = g["input"].astype(np.float32).reshape(B, C, YX)
    h = g["h_cur"].astype(np.float32).reshape(B, E, YX)
    c = g["c_cur"].astype(np.float32).reshape(B, E, YX)
    ck = g["concat_k"].astype(np.float32)
    cv = g["concat_v"].astype(np.float32)

    in_maps = []
    for i in range(NC_):
        s = slice(i * BL, (i + 1) * BL)
        m = dict(shared)
        m.update(x=np.ascontiguousarray(x[s]), h=np.ascontiguousarray(h[s]),
                 cc=np.ascontiguousarray(c[s]), ck=np.ascontiguousarray(ck[s]),
                 cv=np.ascontiguousarray(cv[s]), mp=np.ascontiguousarray(mp[s]))
        in_maps.append(m)
    return in_maps, ln_trivial


def _run(in_maps, ln_trivial):
    nc = _get_nc(ln_trivial)
    return run_bass_kernel_spmd(nc, in_maps, core_ids=list(range(NC_)))


def kernel(**inputs):
    in_maps, ln_trivial = _prep(inputs)
    res = _run(in_maps, ln_trivial)
    h = np.concatenate([r["h_next"] for r in res.results], axis=0).reshape(B, E, H, W)
    c = np.concatenate([r["c_next"] for r in res.results], axis=0).reshape(B, E, H, W)
    k = np.concatenate([r["k_out"] for r in res.results], axis=0)
    v = np.concatenate([r["v_out"] for r in res.results], axis=0)
    return h, c, k, v
